# revision 1
# baseline (speedup 1.0000x reference)
"""DetectiveNN Trainium2 kernel: two 2-layer bidirectional LSTM stacks.

Strategy: 6 NeuronCores, one (stack, dir) scan unit per core, 64 streams each:
  core0 rnn fwd, core1 rnn bwd, core2 rnnp-party0 fwd, core3 rnnp-party0 bwd,
  core4 rnnp-party1 fwd, core5 rnnp-party1 bwd.
Backward units receive time-reversed inputs so every core runs the same
forward-scan program (SPMD).  Everything on-device lives in a "gate
transposed" layout: the gate/feature dimension is on SBUF partitions and the
64 streams are the free dim, so h is consumed as the matmul moving operand
and produced already transposed - no per-step transposes.  The input
projection xg = Wih @ x + b is a big GEMM done on-device per layer; xg is
injected into the recurrent PSUM accumulation through an identity stationary
chunk.  L0->L1 handoff is an in-kernel pairwise AllGather.
Speaker compaction and scatter-back are host-side numpy (pure indexing).
"""

import dataclasses
import os

import ml_dtypes
import numpy as np

T, B, D, H, P = 256, 64, 1024, 512, 2
S = 64          # streams per unit
DK = 8          # Din chunks of 128 (1024/128)
HK = 4          # H chunks of 128 (512/128)
MC = 16         # gate chunks of 128 (2048/128)
NCORE = 6
BF16 = ml_dtypes.bfloat16

_CACHE = {}


def _safe_tc(tile_mod, bass_rust):
    """TileContext whose tail drain splits sem waits one per instruction
    (this walrus build rejects any Drain carrying >1 sync wait)."""
    from concourse.vector_clock import ScopedClock

    class SafeTC(tile_mod.TileContext):
        def _drain_and_barrier(self, tick_clock, wait_clock):
            drain_inst = self.nc.sync.drain()
            wait_clock.add_sem_waits(
                drain_inst.ins, ScopedClock({None: tick_clock.global_clock})
            )
            di = drain_inst.ins
            if di.sync_info is None:
                self.nc.all_engine_barrier()
                popped = self.nc._tile_sem_poison_stack.pop()
                assert popped is self._sem_poison
                self.nc.clear_and_free_semaphores(
                    list(self.sems.allocated().values())
                )
                self.nc.all_engine_barrier()
                return
            waits = list(di.sync_info.on_wait)
            ups = list(di.sync_info.on_update)
            if len(waits) > 1:
                di.sync_info = bass_rust.SyncInfo(on_wait=[waits[0]], on_update=ups)
                for w in waits[1:]:
                    d2 = self.nc.sync.drain()
                    d2.ins.sync_info = bass_rust.SyncInfo(on_wait=[w], on_update=[])
            self.nc.all_engine_barrier()
            popped = self.nc._tile_sem_poison_stack.pop()
            assert popped is self._sem_poison
            self.nc.clear_and_free_semaphores(list(self.sems.allocated().values()))
            self.nc.all_engine_barrier()

    return SafeTC


def _rev_t(ap, t_dim_in_ap, t_stride):
    """Flip the sign of the t step in an already-sliced AP (reversed read)."""
    aps = [list(p) for p in ap.ap]
    assert aps[t_dim_in_ap][0] == t_stride, (aps, t_dim_in_ap, t_stride)
    cnt = aps[t_dim_in_ap][1]
    aps[t_dim_in_ap][0] = -t_stride
    return dataclasses.replace(ap, offset=ap.offset + (cnt - 1) * t_stride, ap=aps)


def _split_waits(nc, mybir, limit=1):
    """This walrus build rejects instructions carrying more than one sync
    wait.  Spill excess waits onto no-op absorber instructions inserted just
    before the offender (same engine, same basic block -> same semantics)."""
    for f in nc.m.functions:
        for bb in f.blocks:
            il = bb.instructions
            out = []
            changed = False
            for inst in il:
                si = inst.sync_info
                if si is not None and len(si.on_wait) > limit:
                    waits = list(si.on_wait)
                    for w in waits[:-limit] if limit else waits:
                        out.append(mybir.InstNoOp(
                            name=nc.get_next_instruction_name(),
                            engine=inst.engine,
                            sync_info=mybir.SyncInfo(on_wait=[w], on_update=[]),
                            bass_nofuse=True,
                        ))
                    inst.sync_info = mybir.SyncInfo(
                        on_wait=waits[-limit:] if limit else [],
                        on_update=list(si.on_update),
                    )
                    changed = True
                out.append(inst)
            if changed:
                bb.instructions = out


def _loop(tc, lo, hi, step, unroll):
    """Either a hardware For_i loop or a Python unrolled loop (sim timing)."""
    from contextlib import contextmanager

    if unroll:
        @contextmanager
        def _it(i):
            yield i
        return [_it(i) for i in range(lo, hi, step)]
    return [tc.For_i(lo, hi, step, staggered_reset=True)]


def _build_ip(tc, nc, bass, mybir, ctx, wih_sb, bias_sb, xg, xT=None, gath=None,
              va=None, vb=None, t_steps=T, unroll=False):
    """Input projection: xg[:, t, mc, :] = Wih @ x_t + b for all tokens."""
    dt = mybir.dt
    rhs_pool = ctx.enter_context(tc.tile_pool(name="ip_rhs", bufs=2))
    ps_pool = ctx.enter_context(tc.tile_pool(name="ip_ps", bufs=2, space="PSUM"))
    st_pool = ctx.enter_context(tc.tile_pool(name="ip_st", bufs=3))

    for loop_cm in _loop(tc, 0, t_steps, 4, unroll):
      with loop_cm as i:
        rhs = rhs_pool.tile([128, DK, 4, S], dt.bfloat16)
        if xT is not None:
            tok = (i * S) if unroll else nc.snap(i * S)
            nc.sync.dma_start(
                out=rhs[:, :, :, :],
                in_=xT.rearrange("k p n -> p k n")[:, :, bass.ds(tok, 4 * S)],
            )
        else:
            # L1: gath variants: 0=slot0, 1=slot1, 2=slot0 reversed,
            # 3=slot1 reversed.  va/vb (host data) pick this core's local-time
            # source for the fwd-half / bwd-half input chunks.
            src_a = gath[bass.ds(va, 1), :, :, :, :][0].rearrange(
                "k p t j -> p k t j")[:, :, bass.ds(i, 4), :]
            nc.sync.dma_start(out=rhs[:, 0:HK, :, :], in_=src_a)
            src_b = gath[bass.ds(vb, 1), :, :, :, :][0].rearrange(
                "k p t j -> p k t j")[:, :, bass.ds(i, 4), :]
            nc.scalar.dma_start(out=rhs[:, HK : 2 * HK, :, :], in_=src_b)
        for half in range(2):
            ps = ps_pool.tile([128, 8, 4 * S], dt.float32, space="PSUM")
            for m8 in range(8):
                mc = half * 8 + m8
                for kc in range(DK):
                    nc.tensor.matmul(
                        ps[:, m8, :],
                        wih_sb[:, kc, mc, :],
                        rhs[:, kc, :, :],
                        start=(kc == 0 and m8 % 2 == 0),
                        stop=(kc == DK - 1 and m8 % 2 == 1),
                    )
            st = st_pool.tile([128, 4, 8, S], dt.bfloat16)
            for m8 in range(8):
                mc = half * 8 + m8
                nc.vector.tensor_scalar(
                    st[:, :, m8, :],
                    ps[:, m8, :].rearrange("p (t j) -> p t j", t=4),
                    bias_sb[:, mc : mc + 1],
                    None,
                    mybir.AluOpType.add,
                )
            nc.sync.dma_start(
                out=xg[:, :, half, :][:, bass.ds(i, 4), :],
                in_=st[:, :, :, :].rearrange("p t m j -> p t (m j)"),
            )


def _build_scan(tc, nc, bass, mybir, ctx, whh_sb, ident_sb, xg, hT_store,
                f32_store, t_steps=T, unroll=False):
    dt = mybir.dt
    A = mybir.ActivationFunctionType
    xg_pool = ctx.enter_context(tc.tile_pool(name="sc_xg", bufs=3))
    ps_pool = ctx.enter_context(tc.tile_pool(name="sc_ps", bufs=2, space="PSUM"))
    act_pool = ctx.enter_context(tc.tile_pool(name="sc_act", bufs=2))
    tmp_pool = ctx.enter_context(tc.tile_pool(name="sc_tmp", bufs=2))
    st_pool = ctx.enter_context(tc.tile_pool(name="sc_state", bufs=1))

    h_sb = st_pool.tile([128, HK, S], dt.bfloat16, name="h_state")
    c_sb = st_pool.tile([128, HK, S], dt.float32, name="c_state")
    nc.vector.memset(h_sb[:, :, :], 0.0)
    nc.vector.memset(c_sb[:, :, :], 0.0)

    for loop_cm in _loop(tc, 0, t_steps, 1, unroll):
      with loop_cm as t:
        xgt = xg_pool.tile([128, MC * S], dt.bfloat16)
        nc.sync.dma_start(
            out=xgt[:, :],
            in_=xg[:, bass.ds(t, 1), :, :].rearrange("p a b c -> p (a b c)"),
        )
        g = ps_pool.tile([128, MC, S], dt.float32, space="PSUM")
        for mc in range(MC):
            nc.tensor.matmul(
                g[:, mc, :],
                ident_sb[:, :],
                xgt[:, mc * S : (mc + 1) * S],
                start=(mc in (0, 8)),
                stop=False,
            )
        for mc in range(MC):
            for kc in range(HK):
                nc.tensor.matmul(
                    g[:, mc, :],
                    whh_sb[:, kc, mc, :],
                    h_sb[:, kc, :],
                    start=False,
                    stop=(kc == HK - 1 and mc in (7, 15)),
                )
        act = act_pool.tile([128, MC, S], dt.float32)
        nc.scalar.activation(act[:, 0:8, :], g[:, 0:8, :], A.Sigmoid)
        nc.scalar.activation(act[:, 8:12, :], g[:, 8:12, :], A.Tanh)
        nc.scalar.activation(act[:, 12:16, :], g[:, 12:16, :], A.Sigmoid)
        t1 = tmp_pool.tile([128, HK, S], dt.float32)
        nc.vector.tensor_mul(t1[:, :, :], act[:, 0:4, :], act[:, 8:12, :])
        t2 = tmp_pool.tile([128, HK, S], dt.float32)
        nc.vector.tensor_mul(t2[:, :, :], act[:, 4:8, :], c_sb[:, :, :])
        nc.vector.tensor_add(c_sb[:, :, :], t1[:, :, :], t2[:, :, :])
        tcv = tmp_pool.tile([128, HK, S], dt.float32)
        nc.scalar.activation(tcv[:, :, :], c_sb[:, :, :], A.Tanh)
        hf = tmp_pool.tile([128, HK, S], dt.float32)
        nc.vector.tensor_mul(hf[:, :, :], act[:, 12:16, :], tcv[:, :, :])
        nc.scalar.activation(h_sb[:, :, :], hf[:, :, :], A.Copy)
        if hT_store is not None:
            nc.scalar.dma_start(
                out=hT_store.rearrange("k p t j -> p k t j")[:, :, bass.ds(t, 1), :],
                in_=h_sb[:, :, :].rearrange("p k (t j) -> p k t j", t=1),
            )
        if f32_store is not None:
            nc.scalar.dma_start(
                out=f32_store.rearrange("k p t j -> p k t j")[:, :, bass.ds(t, 1), :],
                in_=hf[:, :, :].rearrange("p k (t j) -> p k t j", t=1),
            )


def build_nc(t_steps=T, n_cores=NCORE, unroll=False):
    import bass_rust
    import concourse.bass as bass
    import concourse.mybir as mybir
    from concourse import tile
    from contextlib import ExitStack

    dt = mybir.dt
    NTOK = t_steps * S
    nc = bass.Bass("TRN2", target_bir_lowering=False, debug=False,
                   num_devices=(1 if unroll else n_cores))

    xT = nc.dram_tensor("xT", [DK, 128, NTOK], dt.bfloat16, kind="ExternalInput").ap()
    wihA = nc.dram_tensor("wihA", [128, DK, MC, 128], dt.bfloat16, kind="ExternalInput").ap()
    whhA = nc.dram_tensor("whhA", [128, HK, MC, 128], dt.bfloat16, kind="ExternalInput").ap()
    biasA = nc.dram_tensor("biasA", [128, MC], dt.float32, kind="ExternalInput").ap()
    wihB = nc.dram_tensor("wihB", [128, DK, MC, 128], dt.bfloat16, kind="ExternalInput").ap()
    whhB = nc.dram_tensor("whhB", [128, HK, MC, 128], dt.bfloat16, kind="ExternalInput").ap()
    biasB = nc.dram_tensor("biasB", [128, MC], dt.float32, kind="ExternalInput").ap()
    ident = nc.dram_tensor("ident", [128, 128], dt.bfloat16, kind="ExternalInput").ap()
    flag = nc.dram_tensor("flag", [1, 2], dt.int32, kind="ExternalInput").ap()
    out_f32 = nc.dram_tensor("out_f32", [HK, 128, t_steps, S], dt.float32,
                             kind="ExternalOutput").ap()

    xg = nc.dram_tensor("xg", [128, t_steps, 2, 8 * S], dt.bfloat16).ap()
    hT0 = nc.dram_tensor("hT0", [HK, 128, t_steps, S], dt.bfloat16).ap()
    gath = nc.dram_tensor("gath", [4, HK, 128, t_steps, S], dt.bfloat16).ap()

    SafeTC = _safe_tc(tile, bass_rust)
    groups = [[2 * k, 2 * k + 1] for k in range(n_cores // 2)]

    with SafeTC(nc) as tc, ExitStack() as ctx:
        cpool = ctx.enter_context(tc.tile_pool(name="const", bufs=1))
        wihA_sb = cpool.tile([128, DK, MC, 128], dt.bfloat16, name="wihA_sb")
        whhA_sb = cpool.tile([128, HK, MC, 128], dt.bfloat16, name="whhA_sb")
        wihB_sb = cpool.tile([128, DK, MC, 128], dt.bfloat16, name="wihB_sb")
        whhB_sb = cpool.tile([128, HK, MC, 128], dt.bfloat16, name="whhB_sb")
        biasA_sb = cpool.tile([128, MC], dt.float32, name="biasA_sb")
        biasB_sb = cpool.tile([128, MC], dt.float32, name="biasB_sb")
        ident_sb = cpool.tile([128, 128], dt.bfloat16, name="ident_sb")
        flag_sb = cpool.tile([1, 2], dt.int32, name="flag_sb")
        for sb, dr in [(wihA_sb, wihA), (whhA_sb, whhA), (wihB_sb, wihB),
                       (whhB_sb, whhB), (biasA_sb, biasA), (biasB_sb, biasB),
                       (ident_sb, ident), (flag_sb, flag)]:
            nc.sync.dma_start(out=sb[...], in_=dr[...])

        if unroll:
            va, vb = 0, 3
        else:
            tmpa = nc.alloc_registers("va_r")
            nc.regs_load(tmpa, flag_sb[0:1, 0:1])
            va = nc.snap(tmpa, donate=True, min_val=0, max_val=3)
            tmpb = nc.alloc_registers("vb_r")
            nc.regs_load(tmpb, flag_sb[0:1, 1:2])
            vb = nc.snap(tmpb, donate=True, min_val=0, max_val=3)

        with ExitStack() as phase:
            _build_ip(tc, nc, bass, mybir, phase, wihA_sb, biasA_sb, xg,
                      xT=xT, t_steps=t_steps, unroll=unroll)
        with ExitStack() as phase:
            _build_scan(tc, nc, bass, mybir, phase, whhA_sb, ident_sb, xg,
                        hT0, None, t_steps=t_steps, unroll=unroll)
        if unroll:
            for v in range(2):
                nc.sync.dma_start(out=gath[v, :, :, :, :], in_=hT0[...])
        else:
            nc.gpsimd.collective_compute(
                "AllGather", mybir.AluOpType.bypass, replica_groups=groups,
                ins=[hT0[...]], outs=[gath[0:2, :, :, :, :]],
            )
        for v in range(2):
            for kc in range(HK):
                nc.sync.dma_start(
                    out=gath[2 + v, kc, :, :, :],
                    in_=gath[v, kc, :, ::-1, :],
                )
        with ExitStack() as phase:
            _build_ip(tc, nc, bass, mybir, phase, wihB_sb, biasB_sb, xg,
                      gath=gath, va=va, vb=vb, t_steps=t_steps, unroll=unroll)
        with ExitStack() as phase:
            _build_scan(tc, nc, bass, mybir, phase, whhB_sb, ident_sb, xg,
                        None, out_f32, t_steps=t_steps, unroll=unroll)
    _split_waits(nc, mybir)
    return nc


# ---------------- host-side data prep ----------------

def _lhsT_tiles(W):
    """W: (4H', Din') -> (128, Din'/128, 4H'/128, 128) [kp, kc, mc, mp] bf16."""
    M, K = W.shape
    t = W.reshape(M // 128, 128, K // 128, 128)   # [mc, mp, kc, kp]
    return np.ascontiguousarray(t.transpose(3, 2, 0, 1)).astype(BF16)


def _unit_inputs(x_unit, t_steps):
    """x_unit: (T, S, Din) fp32 local-time order -> xT (DK,128,T*S) bf16."""
    Din = x_unit.shape[2]
    xt = x_unit.reshape(t_steps * S, Din).T          # (Din, NTOK)
    xt = xt.reshape(Din // 128, 128, t_steps * S)
    return np.ascontiguousarray(xt).astype(BF16)


def _prep_inputs(inputs, t_steps=T):
    U = np.asarray(inputs["U"], np.float32)            # (T, B, D)
    qmask = np.asarray(inputs["qmask"], np.float32)    # (B, T, P)
    U_bt = U.transpose(1, 0, 2)                        # (B, T, D)
    mask = qmask > 0
    pos = np.cumsum(mask.astype(np.int64), axis=1) - 1  # (B, T, P)

    # compaction: party p stream for batch b = utterances with speaker p,
    # packed to the front, zero-padded.
    parties = np.zeros((P, B, t_steps, D), np.float32)
    b_idx, t_idx = np.nonzero(mask[:, :, 0])
    parties[0, b_idx, pos[b_idx, t_idx, 0]] = U_bt[b_idx, t_idx]
    b_idx, t_idx = np.nonzero(mask[:, :, 1])
    parties[1, b_idx, pos[b_idx, t_idx, 1]] = U_bt[b_idx, t_idx]

    # unit inputs, (T, S, D) in unit-local time
    rnn_x = U                                          # (T, B=S, D)
    units = [
        rnn_x,
        rnn_x[::-1],
        parties[0].transpose(1, 0, 2),
        parties[0].transpose(1, 0, 2)[::-1],
        parties[1].transpose(1, 0, 2),
        parties[1].transpose(1, 0, 2)[::-1],
    ]

    def wset(stack, lay, d):
        return (
            _lhsT_tiles(np.asarray(inputs[f"{stack}_Wih{lay}"][d], np.float32)),
            _lhsT_tiles(np.asarray(inputs[f"{stack}_Whh{lay}"][d], np.float32)),
            np.ascontiguousarray(
                np.asarray(inputs[f"{stack}_b{lay}"][d], np.float32)
                .reshape(MC, 128).T
            ),
        )

    stacks = ["rnn", "rnn", "rnnp", "rnnp", "rnnp", "rnnp"]
    ident = np.eye(128, dtype=BF16)
    in_maps = []
    for c in range(NCORE):
        d = c % 2
        wA = wset(stacks[c], 0, d)
        wB = wset(stacks[c], 1, d)
        in_maps.append({
            "xT": _unit_inputs(units[c], t_steps),
            "wihA": wA[0], "whhA": wA[1], "biasA": wA[2],
            "wihB": wB[0], "whhB": wB[1], "biasB": wB[2],
            "ident": ident,
            "flag": np.array([[0, 3] if d == 0 else [2, 1]], np.int32),
        })
    return in_maps, mask, pos


def _assemble(results, mask, pos, t_steps=T):
    # per-core out: (HK, 128, T, S) fp32 -> (T, S, 512) in unit-local time
    outs = []
    for c in range(NCORE):
        o = results[c]["out_f32"].reshape(H, t_steps, S).transpose(1, 2, 0)
        if c % 2 == 1:
            o = o[::-1]                                 # back to global time
        outs.append(o)
    U_s = np.concatenate([outs[0], outs[1]], axis=-1)   # (T, B, 2H)
    E = np.stack([
        np.concatenate([outs[2], outs[3]], axis=-1),
        np.concatenate([outs[4], outs[5]], axis=-1),
    ])                                                  # (P, T, B, 2H)
    E = E.transpose(0, 2, 1, 3)                         # (P, B, T, 2H)

    U_p = np.zeros((B, t_steps, 2 * H), np.float32)
    for p in range(P):
        idx = np.clip(pos[:, :, p], 0, t_steps - 1)
        gathered = np.take_along_axis(E[p], idx[:, :, None], axis=1)
        U_p = np.where(mask[:, :, p][:, :, None], gathered, U_p)
    U_p = U_p.transpose(1, 0, 2)                        # (T, B, 2H)
    return np.concatenate([U_s, U_p], axis=-1).astype(np.float32)


def _get_compiled():
    if "nc" not in _CACHE:
        _CACHE["nc"] = build_nc()
    return _CACHE["nc"]


def kernel(**inputs):
    from concourse.bass_utils import run_bass_kernel_spmd

    nc = _get_compiled()
    in_maps, mask, pos = _prep_inputs(inputs)
    trace = bool(int(os.environ.get("KERNEL_TRACE", "0")))
    res = run_bass_kernel_spmd(nc, in_maps, list(range(NCORE)), trace=trace)
    _CACHE["last_exec_time_ns"] = res.exec_time_ns
    return _assemble(res.results, mask, pos)



# revision 53
# speedup vs baseline: 2.2328x; 2.2328x over previous
"""DetectiveNN Trainium2 kernel: two 2-layer bidirectional LSTM stacks.

V3 layout: 8 NeuronCores, each runs ONE direction for 48 streams:
  16 streams of the `rnn` stack + 32 streams of the `rnnp` stack (the two
  speaker parties share rnnp weights, so their 128 compacted streams merge).
Core pairs (2k, 2k+1) = (fwd, bwd) over the same 48 streams; bwd cores get
time-reversed inputs so every core runs the same forward-scan program (SPMD).

Per layer each core computes its own input projection xg = Wih@x + b (bf16
GEMM) and the recurrent scan.  The IP is CHUNK-INTERLEAVED into the scan
steps: the scan's serial act/vector chain leaves the PE idle, so IP matmuls
fill those gaps, which also keeps the tensor engine in its fast p-state.
The recurrent Whh matmuls use fp8e4 + DoubleRow (two 128-K-chunks per
instruction at 0.5 cyc/row); Wih/Whh/bias are pre-scaled by 32 so fp8 hits a
good range, and the activations descale via their scale operand (1/32).
Gate order is repacked [g, i, f, o] with two PSUM stop-groups per unit so
tanh(g)/sigmoid(i) fire at 50% of the step's matmuls, sigmoid(f,o) at 100%.

L0->L1 handoff: pairwise AllGather of per-step h (bf16) in two t-halves; the
half needed first by the reversed reader ships first.  Partner h is consumed
through a reversed-t access pattern directly - no re-materialized copies.

Speaker compaction and scatter-back are host-side numpy (pure indexing).
"""

import dataclasses
import os

import ml_dtypes
import numpy as np

T, B, D, H, P = 256, 64, 1024, 512, 2
S1, S2, S = 16, 32, 48  # rnn streams, rnnp streams, total per core
DK = 8                  # contraction chunks of 128 (D=1024 and 2H=1024)
HK = 4                  # H chunks of 128
MC = 16                 # gate chunks of 128 (4H=2048)
NCORE = 8
GT = 8                  # t-steps per IP group (= steps per For_i body)
NG = T // GT            # groups per layer
GLEAD = 4               # IP groups computed ahead of the scan
LEAD_T = GLEAD * GT
TH = T // 2             # t-half for the chunked AllGather
WS = 32.0               # fp8 weight prescale
XG_PREF = 3             # xg load lookahead (steps)
BF16 = ml_dtypes.bfloat16
FP8 = ml_dtypes.float8_e4m3fn

_CACHE = {}


def _safe_tc(tile_mod, bass_rust):
    """TileContext whose tail drain splits sem waits one per instruction
    (this walrus build rejects any Drain carrying >1 sync wait)."""
    from concourse.vector_clock import ScopedClock

    class SafeTC(tile_mod.TileContext):
        def _drain_and_barrier(self, tick_clock, wait_clock):
            drain_inst = self.nc.sync.drain()
            wait_clock.add_sem_waits(
                drain_inst.ins, ScopedClock({None: tick_clock.global_clock})
            )
            di = drain_inst.ins
            if di.sync_info is None:
                self.nc.all_engine_barrier()
                popped = self.nc._tile_sem_poison_stack.pop()
                assert popped is self._sem_poison
                self.nc.clear_and_free_semaphores(
                    list(self.sems.allocated().values())
                )
                self.nc.all_engine_barrier()
                return
            waits = list(di.sync_info.on_wait)
            ups = list(di.sync_info.on_update)
            if len(waits) > 1:
                di.sync_info = bass_rust.SyncInfo(on_wait=[waits[0]], on_update=ups)
                for w in waits[1:]:
                    d2 = self.nc.sync.drain()
                    d2.ins.sync_info = bass_rust.SyncInfo(on_wait=[w], on_update=[])
            self.nc.all_engine_barrier()
            popped = self.nc._tile_sem_poison_stack.pop()
            assert popped is self._sem_poison
            self.nc.clear_and_free_semaphores(list(self.sems.allocated().values()))
            self.nc.all_engine_barrier()

    return SafeTC


def _rev_t(ap, t_dim_in_ap):
    """Reverse the t dimension of an AP in place: read last element first."""
    aps = [list(p) for p in ap.ap]
    stride, cnt = aps[t_dim_in_ap]
    aps[t_dim_in_ap][0] = -stride
    return dataclasses.replace(ap, offset=ap.offset + (cnt - 1) * stride, ap=aps)


def _split_waits(nc, mybir, limit=1):
    """This walrus build rejects instructions carrying more than one sync
    wait.  Spill excess waits onto no-op absorber instructions inserted just
    before the offender (same engine, same basic block -> same semantics)."""
    for f in nc.m.functions:
        for bb in f.blocks:
            il = bb.instructions
            out = []
            changed = False
            for inst in il:
                si = inst.sync_info
                if si is not None and len(si.on_wait) > limit:
                    waits = list(si.on_wait)
                    for w in waits[:-limit] if limit else waits:
                        out.append(mybir.InstNoOp(
                            name=nc.get_next_instruction_name(),
                            engine=inst.engine,
                            sync_info=mybir.SyncInfo(on_wait=[w], on_update=[]),
                            bass_nofuse=True,
                        ))
                    inst.sync_info = mybir.SyncInfo(
                        on_wait=waits[-limit:] if limit else [],
                        on_update=list(si.on_update),
                    )
                    changed = True
                out.append(inst)
            if changed:
                bb.instructions = out


def _loop(tc, lo, hi, step, unroll):
    """Either a hardware For_i loop or a Python unrolled loop (sim timing)."""
    from contextlib import contextmanager

    if unroll:
        @contextmanager
        def _it(i):
            yield i
        return [_it(i) for i in range(lo, hi, step)]
    return [tc.For_i(lo, hi, step, staggered_reset=False)]


class _Ctx:
    """Bundle of build-time handles shared by the emit helpers."""


def _emit_rhs_load(C, rhs, tok_base, w1, wS1, wS2):
    """Load rhs [128, DK, GT, S] for the IP group at token tok_base + w.
    For L1 (C.l1_src set): kc 0:4 <- own h (normal time), kc 4:8 <- partner
    h (already reversed by the producer); the L1 Wih K-halves are pre-swapped
    host-side for bwd cores so this layout is direction-independent."""
    nc, bass = C.nc, C.bass
    if C.l1_src is None:
        nc.scalar.dma_start(
            out=rhs[:, :, :, :],
            in_=C.xT.rearrange("k p t j -> p k t j")
            [:, :, tok_base:, :][:, :, bass.ds(w1, GT), :],
        )
    else:
        # own half only; the partner half is consumed straight from the
        # SBUF granule tiles (see C.gran)
        norm_view = C.l1_src
        nc.scalar.dma_start(
            out=rhs[:, 0:HK, :, :],
            in_=norm_view[:, :, bass.ds(w1, GT), :],
        )


def _emit_granule_load(C, g):
    """Load the 64-token partner-h granule g (tokens [64g, 64g+64)) from the
    gathered reversed buffer into SBUF - the only slot-dynamic reads."""
    nc = C.nc
    dt = C.mybir.dt
    tile = C.prt_pool.tile([128, HK, 64, S], dt.bfloat16)
    src, base = (C.gRA_v, 64 * g) if g < 2 else (C.gRB_v, 64 * g - TH)
    nc.scalar.dma_start(out=tile[:, :, :, :], in_=src[:, :, base:base + 64, :])
    C.gran[g] = tile


def _emit_ip_subchunk(C, j, tok_base, w1, wS1, wS2):
    """IP sub-chunk j (of GT=8) for the group at token tok_base + w:
    mc chunks (2j, 2j+1) for both units.  j==0 allocates + loads this
    group's rhs tile (pool bufs pipeline the load across groups)."""
    nc, bass, mybir = C.nc, C.bass, C.mybir
    dt = mybir.dt

    tau = tok_base + w1  # absolute first token of this group (int)
    if j == 0:
        C.ip_rhs = C.rhs_pool.tile([128, DK, GT, S], dt.bfloat16)
        _emit_rhs_load(C, C.ip_rhs, tok_base, w1, wS1, wS2)
        if C.l1_src is not None and (tau + 32) % 64 == 0 and tau + 32 < T:
            _emit_granule_load(C, (tau + 32) // 64)
    rhs = C.ip_rhs
    mco = 2 * j

    for u in (1, 0):
        if u == 1:
            ssl, su, wih_sb, bias_sb = slice(S1, S), S2, C.wih2_sb, C.bias2_sb
        else:
            ssl, su, wih_sb, bias_sb = slice(0, S1), S1, C.wih1_sb, C.bias1_sb
        nfree = GT * su

        if mco % 8 == 0:
            if u == 1:
                C.stg2 = C.stg2_pool.tile([128, GT, 8, S2], dt.bfloat16)
            else:
                C.stg1 = C.stg1_pool.tile([128, GT, 8, S1], dt.bfloat16)
        stg = C.stg2 if u == 1 else C.stg1

        for m2 in range(2):
            mc = mco + m2
            ps = C.ip_ps_pool.tile([128, 512], dt.float32, space="PSUM")
            for kc in range(DK):
                if C.l1_src is not None and kc >= HK:
                    # partner half from the 64-token SBUF granule
                    gran = C.gran[tau // 64]
                    off = tau % 64
                    moving = gran[:, kc - HK, off:off + GT, ssl]
                else:
                    moving = rhs[:, kc, :, ssl]
                nc.tensor.matmul(
                    ps[:, 0:nfree],
                    wih_sb[:, kc, mc, :],
                    moving,
                    start=(kc == 0),
                    stop=(kc == DK - 1),
                )
            if u == 1:
                nc.vector.tensor_scalar(
                    stg[:, :, mc % 8, :],
                    ps[:, 0:nfree].rearrange("p (t j) -> p t j", t=GT),
                    bias_sb[:, mc:mc + 1],
                    None,
                    mybir.AluOpType.add,
                )
            else:
                nc.scalar.activation(
                    stg[:, :, mc % 8, :],
                    ps[:, 0:nfree].rearrange("p (t j) -> p t j", t=GT),
                    mybir.ActivationFunctionType.Identity,
                    bias=bias_sb[:, mc:mc + 1],
                )
        if mco % 8 == 6:  # slab of 8 mc chunks complete -> store
            slab = mco - 6
            xg_u = C.xg2 if u == 1 else C.xg1
            nc.scalar.dma_start(
                out=xg_u[:, tok_base:, slab:slab + 8, :][:, bass.ds(w1, GT), :, :],
                in_=stg[:, :, :, :],
            )


def _emit_scan_step(C, j, tok_base, w1, hdst_view):
    """One scan step for both units; absolute token = tok_base + w + j.
    hdst_view: pre-sliced dram view [128, HK, nt, S] (t offset j applied by
    caller) receiving this step's bf16 h at index w."""
    nc, bass, mybir = C.nc, C.bass, C.mybir
    dt = mybir.dt
    A = mybir.ActivationFunctionType

    if j % 4 == 0:
        # quad xg load: 4 steps per DMA (stays under the 15-symbolic-DMA
        # per-queue-per-body ring limit)
        C.xgt1 = C.xgt1_pool.tile([128, 4, MC, S1], dt.bfloat16)
        C.xgt2 = C.xgt2_pool.tile([128, 4, MC, S2], dt.bfloat16)
        nc.sync.dma_start(
            out=C.xgt2[:, :, :, :],
            in_=C.xg2[:, tok_base + j:, :, :][:, bass.ds(w1, 4), :, :],
        )
        nc.sync.dma_start(
            out=C.xgt1[:, :, :, :],
            in_=C.xg1[:, tok_base + j:, :, :][:, bass.ds(w1, 4), :, :],
        )
    xgt1, xgt2 = C.xgt1, C.xgt2
    hstore = hdst_view

    for u in (1, 0):  # big unit first
        if u == 1:
            ssl, su, whh, xgt = slice(S1, S), S2, C.whh2_sb, xgt2
            gA, gB, act = C.g2A, C.g2B, C.act2
            tg, t1s, t2s, tcv = C.tg2, C.t12, C.t22, C.tc2
        else:
            ssl, su, whh, xgt = slice(0, S1), S1, C.whh1_sb, xgt1
            gA, gB, act = C.g1A, C.g1B, C.act1
            tg, t1s, t2s, tcv = C.tg1, C.t11, C.t21, C.tc1

        # inject xg (bf16, x32-scaled), then accumulate Whh@h fp8 DoubleRow
        for grp, gps in ((0, gA), (1, gB)):
            for m8 in range(8):
                nc.tensor.matmul(
                    gps[:, m8, 0:su],
                    C.ident_sb[:, :],
                    xgt[:, j % 4, grp * 8 + m8, :],
                    start=(m8 == 0),
                    stop=False,
                )
            for m8 in range(8):
                for kp in range(2):
                    nc.tensor.matmul(
                        gps[:, m8, 0:su],
                        whh[:, kp, :, grp * 8 + m8, :],
                        C.h8[:, 2 * kp:2 * kp + 2, ssl],
                        start=False,
                        stop=(m8 == 7 and kp == 1),
                        perf_mode=mybir.MatmulPerfMode.DoubleRow,
                    )
        # group A done -> tanh(g) [chunks 0:4], sigmoid(i) [4:8]
        nc.scalar.activation(tg[:, :, :], gA[:, 0:4, 0:su], A.Tanh, scale=1.0 / WS)
        nc.scalar.activation(act[:, 0:4, :], gA[:, 4:8, 0:su], A.Sigmoid, scale=1.0 / WS)
        # group B done -> sigmoid(f,o) [8:16]
        nc.scalar.activation(act[:, 4:12, :], gB[:, :, 0:su], A.Sigmoid, scale=1.0 / WS)
        # c = sig(f)*c + sig(i)*tanh(g);  h = sig(o)*tanh(c)
        nc.vector.tensor_mul(t1s[:, :, :], act[:, 0:4, :], tg[:, :, :])
        nc.vector.tensor_mul(t2s[:, :, :], act[:, 4:8, :], C.c_sb[:, :, ssl])
        nc.vector.tensor_add(C.c_sb[:, :, ssl], t1s[:, :, :], t2s[:, :, :])
        nc.scalar.activation(tcv[:, :, :], C.c_sb[:, :, ssl], A.Tanh)
        nc.vector.tensor_mul(C.h8[:, :, ssl], act[:, 8:12, :], tcv[:, :, :])
        # bf16 h for the handoff / output, from the fp32 operands (NOT from
        # the fp8 state - fp8 noise here would leak into L1 and the output);
        # 4 rotating t-slots so the paired store never stalls the queues
        nc.vector.tensor_mul(C.hbf[:, :, j % 4, ssl], act[:, 8:12, :], tcv[:, :, :])

    if j % 2 == 1:  # store two steps of h per DMA
        sl = (j - 1) % 4
        nc.gpsimd.dma_start(
            out=hstore[:, :, j - 1:, :][:, :, bass.ds(w1, 2), :],
            in_=C.hbf[:, :, sl:sl + 2, :],
        )


def build_nc(n_cores=NCORE, unroll=False):
    import bass_rust
    import concourse.bass as bass
    import concourse.mybir as mybir
    from concourse import tile
    from contextlib import ExitStack

    dt = mybir.dt
    nc = bass.Bass("TRN2", target_bir_lowering=False, debug=False,
                   num_devices=(1 if unroll else n_cores))

    C = _Ctx()
    C.nc, C.bass, C.mybir = nc, bass, mybir

    C.xT = nc.dram_tensor("xT", [DK, 128, T, S], dt.bfloat16, kind="ExternalInput").ap()
    w_in = {}
    for nm in ("A1", "A2", "B1", "B2"):
        w_in[f"wih{nm}"] = nc.dram_tensor(f"wih{nm}", [128, DK, MC, 128], dt.bfloat16, kind="ExternalInput").ap()
        w_in[f"whh{nm}"] = nc.dram_tensor(f"whh{nm}", [128, 2, 2, MC, 128], dt.float8e4, kind="ExternalInput").ap()
        w_in[f"bias{nm}"] = nc.dram_tensor(f"bias{nm}", [128, MC], dt.float32, kind="ExternalInput").ap()
    ident = nc.dram_tensor("ident", [128, 128], dt.bfloat16, kind="ExternalInput").ap()
    flag = nc.dram_tensor("flag", [1, 4], dt.int32, kind="ExternalInput").ap()
    out = nc.dram_tensor("out", [HK, 128, T, S], dt.bfloat16, kind="ExternalOutput").ap()

    C.xg1 = nc.dram_tensor("xg1", [128, T, MC, S1], dt.bfloat16).ap()
    C.xg2 = nc.dram_tensor("xg2", [128, T, MC, S2], dt.bfloat16).ap()
    hT = nc.dram_tensor("hT", [HK, 128, T, S], dt.bfloat16).ap()
    # own h reversed: hTr_A = reverse(hT[TH:]) (global tokens 255..128),
    # hTr_B = reverse(hT[:TH]); A ships first (the partner needs it first).
    hTr_A = nc.dram_tensor("hTr_A", [HK, 128, TH, S], dt.bfloat16).ap()
    hTr_B = nc.dram_tensor("hTr_B", [HK, 128, TH, S], dt.bfloat16).ap()
    gathR_A = nc.dram_tensor("gathR_A", [2, HK, 128, TH, S], dt.bfloat16).ap()
    gathR_B = nc.dram_tensor("gathR_B", [2, HK, 128, TH, S], dt.bfloat16).ap()

    SafeTC = _safe_tc(tile, bass_rust)
    groups = [[2 * k, 2 * k + 1] for k in range(max(n_cores // 2, 1))]

    with SafeTC(nc) as tc, ExitStack() as ctx:
        cpool = ctx.enter_context(tc.tile_pool(name="const", bufs=1))
        C.wih1_sb = cpool.tile([128, DK, MC, 128], dt.bfloat16, name="wih1_sb")
        C.wih2_sb = cpool.tile([128, DK, MC, 128], dt.bfloat16, name="wih2_sb")
        C.whh1_sb = cpool.tile([128, 2, 2, MC, 128], dt.float8e4, name="whh1_sb")
        C.whh2_sb = cpool.tile([128, 2, 2, MC, 128], dt.float8e4, name="whh2_sb")
        C.bias1_sb = cpool.tile([128, MC], dt.float32, name="bias1_sb")
        C.bias2_sb = cpool.tile([128, MC], dt.float32, name="bias2_sb")
        C.ident_sb = cpool.tile([128, 128], dt.bfloat16, name="ident_sb")
        flag_sb = cpool.tile([1, 4], dt.int32, name="flag_sb")

        def load_layer(nm1, nm2):
            for sb, dr in [(C.wih1_sb, w_in[f"wih{nm1}"]), (C.wih2_sb, w_in[f"wih{nm2}"]),
                           (C.whh1_sb, w_in[f"whh{nm1}"]), (C.whh2_sb, w_in[f"whh{nm2}"]),
                           (C.bias1_sb, w_in[f"bias{nm1}"]), (C.bias2_sb, w_in[f"bias{nm2}"])]:
                nc.sync.dma_start(out=sb[...], in_=dr[...])

        load_layer("A1", "A2")
        nc.sync.dma_start(out=C.ident_sb[...], in_=ident[...])
        nc.sync.dma_start(out=flag_sb[...], in_=flag[...])

        if unroll:
            C.vb = 1
        else:
            tmp = nc.alloc_registers("vb_r")
            nc.regs_load(tmp, flag_sb[0:1, 1:2])
            C.vb = nc.snap(tmp, donate=True, min_val=0, max_val=1)

        spool = ctx.enter_context(tc.tile_pool(name="state", bufs=1))
        C.h8 = spool.tile([128, HK, S], dt.float8e4, name="h8")
        C.hbf = spool.tile([128, HK, 4, S], dt.bfloat16, name="hbf")
        C.c_sb = spool.tile([128, HK, S], dt.float32, name="c_sb")

        def gview(gh, slot):
            """[128, HK, TH, S] view of gath half gh at slot (static)."""
            return gh[slot].rearrange("k p t j -> p k t j")

        def hview(ht):
            return ht.rearrange("k p t j -> p k t j")

        for lay in range(2):
            nm1, nm2 = ("A1", "A2") if lay == 0 else ("B1", "B2")
            with ExitStack() as phase:
                C.rhs_pool = phase.enter_context(tc.tile_pool(name=f"rhs{lay}", bufs=2))
                C.ip_ps_pool = phase.enter_context(tc.tile_pool(name=f"ipps{lay}", bufs=2, space="PSUM"))
                C.stg1_pool = phase.enter_context(tc.tile_pool(name=f"s1p{lay}", bufs=2))
                C.stg2_pool = phase.enter_context(tc.tile_pool(name=f"s2p{lay}", bufs=2))
                C.xgt1_pool = phase.enter_context(tc.tile_pool(name=f"x1p{lay}", bufs=4))
                C.xgt2_pool = phase.enter_context(tc.tile_pool(name=f"x2p{lay}", bufs=4))
                C.prt_pool = phase.enter_context(tc.tile_pool(name=f"prt{lay}", bufs=2))
                gpool = phase.enter_context(tc.tile_pool(name=f"g{lay}", bufs=1, space="PSUM"))
                apool = phase.enter_context(tc.tile_pool(name=f"act{lay}", bufs=1))

                C.g1A = gpool.tile([128, 8, 64], dt.float32, name=f"g1A{lay}", space="PSUM")
                C.g1B = gpool.tile([128, 8, 64], dt.float32, name=f"g1B{lay}", space="PSUM")
                C.g2A = gpool.tile([128, 8, 64], dt.float32, name=f"g2A{lay}", space="PSUM")
                C.g2B = gpool.tile([128, 8, 64], dt.float32, name=f"g2B{lay}", space="PSUM")
                C.act1 = apool.tile([128, 12, S1], dt.float32, name=f"a1{lay}")
                C.act2 = apool.tile([128, 12, S2], dt.float32, name=f"a2{lay}")
                C.tg1 = apool.tile([128, HK, S1], dt.float32, name=f"tg1{lay}")
                C.tg2 = apool.tile([128, HK, S2], dt.float32, name=f"tg2{lay}")
                C.t11 = apool.tile([128, HK, S1], dt.float32, name=f"t11{lay}")
                C.t12 = apool.tile([128, HK, S2], dt.float32, name=f"t12{lay}")
                C.t21 = apool.tile([128, HK, S1], dt.float32, name=f"t21{lay}")
                C.t22 = apool.tile([128, HK, S2], dt.float32, name=f"t22{lay}")
                C.tc1 = apool.tile([128, HK, S1], dt.float32, name=f"tc1{lay}")
                C.tc2 = apool.tile([128, HK, S2], dt.float32, name=f"tc2{lay}")

                if lay == 1:
                    load_layer(nm1, nm2)
                nc.vector.memset(C.h8[:, :, :], 0.0)
                nc.vector.memset(C.hbf[:, :, :, :], 0.0)
                nc.vector.memset(C.c_sb[:, :, :], 0.0)

                # segments: (nsteps, tok_base, l1 normal view, l1 rev view,
                #            h-store view, do_ip)
                if lay == 0:
                    segs = [
                        (T - LEAD_T, 0, None, hview(hT), True),
                        (LEAD_T, T - LEAD_T, None,
                         hview(hT)[:, :, T - LEAD_T:, :], False),
                    ]
                else:
                    # partner h (already reversed to global 255-tau order)
                    C.gRA_v = gathR_A[bass.ds(C.vb, 1), :, :, :, :][0] \
                        .rearrange("k p t j -> p k t j")
                    C.gRB_v = gathR_B[bass.ds(C.vb, 1), :, :, :, :][0] \
                        .rearrange("k p t j -> p k t j")
                    C.gran = {}
                    segs = [
                        (96, 0, hview(hT)[:, :, LEAD_T:, :], hview(out), True),
                        (128, 96, hview(hT)[:, :, TH:, :],
                         hview(out)[:, :, 96:, :], True),
                        (LEAD_T, T - LEAD_T, None,
                         hview(out)[:, :, T - LEAD_T:, :], False),
                    ]

                # IP lead groups (static indices; unsliced views)
                if lay == 0:
                    C.l1_src = None
                else:
                    C.l1_src = hview(hT)
                    _emit_granule_load(C, 0)
                for g in range(GLEAD):
                    base = g * GT
                    for j in range(GT):
                        _emit_ip_subchunk(C, j, 0, base, base * S1, base * S2)

                for (nsteps, tok_base, l1s, hv, do_ip) in segs:
                    C.l1_src = l1s
                    for i in range(0, nsteps, GT):
                        for j in range(GT):
                            if do_ip:
                                _emit_ip_subchunk(
                                    C, j, tok_base + LEAD_T, i, i * S1, i * S2)
                            _emit_scan_step(C, j, tok_base, i, hv)

                if lay == 0:
                    # reverse own h locally (per k-chunk: 3-dim DMAs), then
                    # AllGather; the A half (global 255..128) ships first
                    # because the partner's L1-IP consumes it first.
                    for kc in range(HK):
                        nc.sync.dma_start(
                            out=hTr_A[kc, :, :, :],
                            in_=_rev_t(hT[kc, :, TH:, :], 1),
                        )
                    for kc in range(HK):
                        nc.scalar.dma_start(
                            out=hTr_B[kc, :, :, :],
                            in_=_rev_t(hT[kc, :, 0:TH, :], 1),
                        )
                    if unroll:
                        for g, hsrc in ((gathR_A, hTr_A), (gathR_B, hTr_B)):
                            for v in range(2):
                                nc.sync.dma_start(out=g[v, :, :, :, :], in_=hsrc[...])
                    else:
                        nc.gpsimd.collective_compute(
                            "AllGather", mybir.AluOpType.bypass,
                            replica_groups=groups,
                            ins=[hTr_A[...]], outs=[gathR_A[0:2, :, :, :, :]],
                        )
                        nc.gpsimd.collective_compute(
                            "AllGather", mybir.AluOpType.bypass,
                            replica_groups=groups,
                            ins=[hTr_B[...]], outs=[gathR_B[0:2, :, :, :, :]],
                        )
    _split_waits(nc, mybir)
    return nc


# ---------------- host-side data prep ----------------

GATE_PERM = (2, 0, 1, 3)  # new block order [g, i, f, o] from [i, f, g, o]


def _reorder_gates(W):
    """W: (4H, ...) -> gate blocks reordered to [g, i, f, o]."""
    blocks = W.reshape(4, H, *W.shape[1:])
    return np.concatenate([blocks[p] for p in GATE_PERM], axis=0)


def _wih_tiles(W, swap_k_halves=False):
    """(2048, K) fp32 -> [128(kp), K/128, MC, 128(mp)] bf16, x32 scaled.
    swap_k_halves: put columns 512:1024 first (bwd cores' L1 weights: the
    rhs always loads own-direction h into kc 0:4)."""
    Wr = _reorder_gates(W) * WS
    if swap_k_halves:
        Wr = np.concatenate([Wr[:, H:], Wr[:, :H]], axis=1)
    M, K = Wr.shape
    t = Wr.reshape(MC, 128, K // 128, 128)       # [mc, mp, kc, kp]
    return np.ascontiguousarray(t.transpose(3, 2, 0, 1)).astype(BF16)


def _whh_dr_tiles(W):
    """(2048, 512) fp32 -> [128(kp), 2(pair), 2(j), MC, 128(mp)] fp8, x32."""
    Wr = _reorder_gates(W) * WS
    t = Wr.reshape(MC, 128, 4, 128)              # [mc, mp, kchunk, kp]
    t = t.reshape(MC, 128, 2, 2, 128)            # [mc, mp, pair, j, kp]
    return np.ascontiguousarray(t.transpose(4, 2, 3, 0, 1)).astype(FP8)


def _bias_tiles(b):
    br = _reorder_gates(b) * WS
    return np.ascontiguousarray(br.reshape(MC, 128).T.astype(np.float32))


def _core_inputs(u1, u2):
    """u1 (T,S1,D), u2 (T,S2,D) fp32 local time -> xT (DK, 128, T, S) bf16."""
    x48 = np.concatenate([u1, u2], axis=1)          # (T, S, D)
    xt = x48.transpose(2, 0, 1)                     # (D, T, S)
    return np.ascontiguousarray(xt.reshape(DK, 128, T, S)).astype(BF16)


def _prep_inputs(inputs):
    U = np.asarray(inputs["U"], np.float32)            # (T, B, D)
    qmask = np.asarray(inputs["qmask"], np.float32)    # (B, T, P)
    U_bt = U.transpose(1, 0, 2)
    mask = qmask > 0
    pos = np.cumsum(mask.astype(np.int64), axis=1) - 1

    parties = np.zeros((P, B, T, D), np.float32)
    for p in range(P):
        b_idx, t_idx = np.nonzero(mask[:, :, p])
        parties[p, b_idx, pos[b_idx, t_idx, p]] = U_bt[b_idx, t_idx]
    partiesM = parties.reshape(P * B, T, D).transpose(1, 0, 2)  # (T, 128, D)

    def wset(stack, lay, d):
        return (
            _wih_tiles(np.asarray(inputs[f"{stack}_Wih{lay}"][d], np.float32),
                       swap_k_halves=(lay == 1 and d == 1)),
            _whh_dr_tiles(np.asarray(inputs[f"{stack}_Whh{lay}"][d], np.float32)),
            _bias_tiles(np.asarray(inputs[f"{stack}_b{lay}"][d], np.float32)),
        )

    wsets = {(st, la, d): wset(st, la, d)
             for st in ("rnn", "rnnp") for la in (0, 1) for d in (0, 1)}
    ident_np = np.eye(128, dtype=BF16)

    in_maps = []
    for c in range(NCORE):
        k, d = c // 2, c % 2
        u1 = U[:, 16 * k:16 * k + 16, :]
        u2 = partiesM[:, 32 * k:32 * k + 32, :]
        if d == 1:
            u1, u2 = u1[::-1], u2[::-1]
        m = {
            "xT": _core_inputs(u1, u2),
            "ident": ident_np,
            # [unused, partner-slot, cond_slot0=(slot==0), cond_slot1]
            "flag": np.array([[0, 1 - d, d, 1 - d]], np.int32),
        }
        for la, nm in ((0, "A"), (1, "B")):
            for ui, st in ((1, "rnn"), (2, "rnnp")):
                wih, whh, bias = wsets[(st, la, d)]
                m[f"wih{nm}{ui}"] = wih
                m[f"whh{nm}{ui}"] = whh
                m[f"bias{nm}{ui}"] = bias
        in_maps.append(m)
    return in_maps, mask, pos


def _assemble(results, mask, pos):
    # per-core out: (HK, 128, T, S) bf16; feature dim on (HK,128)=512
    o = []
    for c in range(NCORE):
        oc = np.asarray(results[c]["out"]).astype(np.float32)
        oc = oc.reshape(H, T, S).transpose(1, 2, 0)    # (T, S, 512) local time
        if c % 2 == 1:
            oc = oc[::-1]
        o.append(oc)

    U_s = np.zeros((T, B, 2 * H), np.float32)
    E = np.zeros((P, B, T, 2 * H), np.float32)
    for k in range(4):
        fwd, bwd = o[2 * k], o[2 * k + 1]
        U_s[:, 16 * k:16 * k + 16, 0:H] = fwd[:, 0:S1]
        U_s[:, 16 * k:16 * k + 16, H:2 * H] = bwd[:, 0:S1]
        for i in range(S2):
            ms = 32 * k + i
            p, b = divmod(ms, B)
            E[p, b, :, 0:H] = fwd[:, S1 + i]
            E[p, b, :, H:2 * H] = bwd[:, S1 + i]

    U_p = np.zeros((B, T, 2 * H), np.float32)
    for p in range(P):
        idx = np.clip(pos[:, :, p], 0, T - 1)
        gathered = np.take_along_axis(E[p], idx[:, :, None], axis=1)
        U_p = np.where(mask[:, :, p][:, :, None], gathered, U_p)
    U_p = U_p.transpose(1, 0, 2)
    return np.concatenate([U_s, U_p], axis=-1).astype(np.float32)


def _get_compiled():
    if "nc" not in _CACHE:
        _CACHE["nc"] = build_nc()
    return _CACHE["nc"]


def kernel(**inputs):
    from concourse.bass_utils import run_bass_kernel_spmd

    nc = _get_compiled()
    in_maps, mask, pos = _prep_inputs(inputs)
    trace = bool(int(os.environ.get("KERNEL_TRACE", "0")))
    res = run_bass_kernel_spmd(nc, in_maps, list(range(NCORE)), trace=trace)
    _CACHE["last_exec_time_ns"] = res.exec_time_ns
    return _assemble(res.results, mask, pos)


# revision 56
# speedup vs baseline: 2.4509x; 1.0977x over previous
"""DetectiveNN Trainium2 kernel: two 2-layer bidirectional LSTM stacks.

V3 layout: 8 NeuronCores, each runs ONE direction for 48 streams:
  16 streams of the `rnn` stack + 32 streams of the `rnnp` stack (the two
  speaker parties share rnnp weights, so their 128 compacted streams merge).
Core pairs (2k, 2k+1) = (fwd, bwd) over the same 48 streams; bwd cores get
time-reversed inputs so every core runs the same forward-scan program (SPMD).

Per layer each core computes its own input projection xg = Wih@x + b (bf16
GEMM) and the recurrent scan.  The IP is CHUNK-INTERLEAVED into the scan
steps: the scan's serial act/vector chain leaves the PE idle, so IP matmuls
fill those gaps, which also keeps the tensor engine in its fast p-state.
The recurrent Whh matmuls use fp8e4 + DoubleRow (two 128-K-chunks per
instruction at 0.5 cyc/row); Wih/Whh/bias are pre-scaled by 32 so fp8 hits a
good range, and the activations descale via their scale operand (1/32).
Gate order is repacked [g, i, f, o] with two PSUM stop-groups per unit so
tanh(g)/sigmoid(i) fire at 50% of the step's matmuls, sigmoid(f,o) at 100%.

L0->L1 handoff: pairwise AllGather of per-step h (bf16) in two t-halves; the
half needed first by the reversed reader ships first.  Partner h is consumed
through a reversed-t access pattern directly - no re-materialized copies.

Speaker compaction and scatter-back are host-side numpy (pure indexing).
"""

import dataclasses
import os

import ml_dtypes
import numpy as np

T, B, D, H, P = 256, 64, 1024, 512, 2
S1, S2, S = 16, 32, 48  # rnn streams, rnnp streams, total per core
DK = 8                  # contraction chunks of 128 (D=1024 and 2H=1024)
HK = 4                  # H chunks of 128
MC = 16                 # gate chunks of 128 (4H=2048)
NCORE = 8
GT = 8                  # t-steps per IP group (= steps per For_i body)
NG = T // GT            # groups per layer
GLEAD = 4               # IP groups computed ahead of the scan
LEAD_T = GLEAD * GT
TH = T // 2             # t-half for the chunked AllGather
WS = 32.0               # fp8 weight prescale
XG_PREF = 3             # xg load lookahead (steps)
BF16 = ml_dtypes.bfloat16
FP8 = ml_dtypes.float8_e4m3fn

_CACHE = {}


def _safe_tc(tile_mod, bass_rust):
    """TileContext whose tail drain splits sem waits one per instruction
    (this walrus build rejects any Drain carrying >1 sync wait)."""
    from concourse.vector_clock import ScopedClock

    class SafeTC(tile_mod.TileContext):
        def _drain_and_barrier(self, tick_clock, wait_clock):
            drain_inst = self.nc.sync.drain()
            wait_clock.add_sem_waits(
                drain_inst.ins, ScopedClock({None: tick_clock.global_clock})
            )
            di = drain_inst.ins
            if di.sync_info is None:
                self.nc.all_engine_barrier()
                popped = self.nc._tile_sem_poison_stack.pop()
                assert popped is self._sem_poison
                self.nc.clear_and_free_semaphores(
                    list(self.sems.allocated().values())
                )
                self.nc.all_engine_barrier()
                return
            waits = list(di.sync_info.on_wait)
            ups = list(di.sync_info.on_update)
            if len(waits) > 1:
                di.sync_info = bass_rust.SyncInfo(on_wait=[waits[0]], on_update=ups)
                for w in waits[1:]:
                    d2 = self.nc.sync.drain()
                    d2.ins.sync_info = bass_rust.SyncInfo(on_wait=[w], on_update=[])
            self.nc.all_engine_barrier()
            popped = self.nc._tile_sem_poison_stack.pop()
            assert popped is self._sem_poison
            self.nc.clear_and_free_semaphores(list(self.sems.allocated().values()))
            self.nc.all_engine_barrier()

    return SafeTC


def _rev_t(ap, t_dim_in_ap):
    """Reverse the t dimension of an AP in place: read last element first."""
    aps = [list(p) for p in ap.ap]
    stride, cnt = aps[t_dim_in_ap]
    aps[t_dim_in_ap][0] = -stride
    return dataclasses.replace(ap, offset=ap.offset + (cnt - 1) * stride, ap=aps)


def _split_waits(nc, mybir, limit=1):
    """This walrus build rejects instructions carrying more than one sync
    wait.  Spill excess waits onto no-op absorber instructions inserted just
    before the offender (same engine, same basic block -> same semantics)."""
    for f in nc.m.functions:
        for bb in f.blocks:
            il = bb.instructions
            out = []
            changed = False
            for inst in il:
                si = inst.sync_info
                if si is not None and len(si.on_wait) > limit:
                    waits = list(si.on_wait)
                    for w in waits[:-limit] if limit else waits:
                        out.append(mybir.InstNoOp(
                            name=nc.get_next_instruction_name(),
                            engine=inst.engine,
                            sync_info=mybir.SyncInfo(on_wait=[w], on_update=[]),
                            bass_nofuse=True,
                        ))
                    inst.sync_info = mybir.SyncInfo(
                        on_wait=waits[-limit:] if limit else [],
                        on_update=list(si.on_update),
                    )
                    changed = True
                out.append(inst)
            if changed:
                bb.instructions = out


def _loop(tc, lo, hi, step, unroll):
    """Either a hardware For_i loop or a Python unrolled loop (sim timing)."""
    from contextlib import contextmanager

    if unroll:
        @contextmanager
        def _it(i):
            yield i
        return [_it(i) for i in range(lo, hi, step)]
    return [tc.For_i(lo, hi, step, staggered_reset=False)]


class _Ctx:
    """Bundle of build-time handles shared by the emit helpers."""


def _emit_rhs_load(C, rhs, tok_base, w1, wS1, wS2):
    """Load rhs [128, DK, GT, S] for the IP group at token tok_base + w.
    For L1 (C.l1_src set): kc 0:4 <- own h (normal time), kc 4:8 <- partner
    h (already reversed by the producer); the L1 Wih K-halves are pre-swapped
    host-side for bwd cores so this layout is direction-independent."""
    nc, bass = C.nc, C.bass
    if C.l1_src is None:
        nc.sync.dma_start(
            out=rhs[:, :, :, :],
            in_=C.xT.rearrange("k p t j -> p k t j")
            [:, :, tok_base:, :][:, :, bass.ds(w1, GT), :],
        )
    else:
        # own half only; the partner half is consumed straight from the
        # SBUF granule tiles (see C.gran)
        norm_view = C.l1_src
        nc.sync.dma_start(
            out=rhs[:, 0:HK, :, :],
            in_=norm_view[:, :, bass.ds(w1, GT), :],
        )


def _emit_granule_load(C, g):
    """Load the 64-token partner-h granule g (tokens [64g, 64g+64)) from the
    gathered reversed buffer into SBUF - the only slot-dynamic reads."""
    nc = C.nc
    dt = C.mybir.dt
    tile = C.prt_pool.tile([128, HK, 64, S], dt.bfloat16)
    src, base = (C.gRA_v, 64 * g) if g < 2 else (C.gRB_v, 64 * g - TH)
    nc.scalar.dma_start(out=tile[:, :, :, :], in_=src[:, :, base:base + 64, :])
    C.gran[g] = tile


def _emit_ip_subchunk(C, j, tok_base, w1, wS1, wS2):
    """IP sub-chunk j (of GT=8) for the group at token tok_base + w:
    mc chunks (2j, 2j+1) for both units.  j==0 allocates + loads this
    group's rhs tile (pool bufs pipeline the load across groups)."""
    nc, bass, mybir = C.nc, C.bass, C.mybir
    dt = mybir.dt

    tau = tok_base + w1  # absolute first token of this group (int)
    if j == 0:
        C.ip_rhs = C.rhs_pool.tile([128, DK, GT, S], dt.bfloat16)
        _emit_rhs_load(C, C.ip_rhs, tok_base, w1, wS1, wS2)
        if C.l1_src is not None and (tau + 32) % 64 == 0 and tau + 32 < T:
            _emit_granule_load(C, (tau + 32) // 64)
    rhs = C.ip_rhs
    mco = 2 * j

    for u in (1, 0):
        if u == 1:
            ssl, su, wih_sb = slice(S1, S), S2, C.wih2_sb
        else:
            ssl, su, wih_sb = slice(0, S1), S1, C.wih1_sb
        nfree = GT * su

        for m2 in range(2):
            mc = mco + m2
            ps = C.ip_ps_pool.tile([128, 512], dt.float32, space="PSUM")
            for kc in range(DK):
                if C.l1_src is not None and kc >= HK:
                    # partner half from the 64-token SBUF granule
                    gran = C.gran[tau // 64]
                    off = tau % 64
                    moving = gran[:, kc - HK, off:off + GT, ssl]
                else:
                    moving = rhs[:, kc, :, ssl]
                nc.tensor.matmul(
                    ps[:, 0:nfree],
                    wih_sb[:, kc, mc, :],
                    moving,
                    start=(kc == 0),
                    stop=(kc == DK - 1),
                )
            C.pending_st.append((u, mc, ps, tok_base, w1))


def _emit_ip_sts(C):
    """Drain pending IP psum->xg stages (bias add, bf16, slab store).
    Emitted after the scan-step chain so these never delay it in the
    ACT/DVE queues."""
    nc, bass, mybir = C.nc, C.bass, C.mybir
    dt = mybir.dt
    for (u, mc, ps, tok_base, w1) in C.pending_st:
        if u == 1:
            su, bias_sb = S2, C.bias2_sb
        else:
            su, bias_sb = S1, C.bias1_sb
        nfree = GT * su
        if mc % 8 == 0:
            if u == 1:
                C.stg2 = C.stg2_pool.tile([128, GT, 8, S2], dt.bfloat16)
            else:
                C.stg1 = C.stg1_pool.tile([128, GT, 8, S1], dt.bfloat16)
        stg = C.stg2 if u == 1 else C.stg1
        if u == 1:
            nc.vector.tensor_scalar(
                stg[:, :, mc % 8, :],
                ps[:, 0:nfree].rearrange("p (t j) -> p t j", t=GT),
                bias_sb[:, mc:mc + 1],
                None,
                mybir.AluOpType.add,
            )
        else:
            nc.scalar.activation(
                stg[:, :, mc % 8, :],
                ps[:, 0:nfree].rearrange("p (t j) -> p t j", t=GT),
                mybir.ActivationFunctionType.Identity,
                bias=bias_sb[:, mc:mc + 1],
            )
        if mc % 8 == 7:  # slab of 8 mc chunks complete -> store
            slab = mc - 7
            xg_u = C.xg2 if u == 1 else C.xg1
            nc.sync.dma_start(
                out=xg_u[:, tok_base:, slab:slab + 8, :][:, bass.ds(w1, GT), :, :],
                in_=stg[:, :, :, :],
            )
    C.pending_st = []


def _emit_scan_pre(C, j, tok_base, w1):
    """xg loads + PSUM-claiming ident injections for step tok_base + w + j.
    These have no dependence on h, so they run while the previous step's
    activation chain completes."""
    nc, bass, mybir = C.nc, C.bass, C.mybir
    dt = mybir.dt

    if j % 4 == 0:
        # quad xg load: 4 steps per DMA
        C.xgt1 = C.xgt1_pool.tile([128, 4, MC, S1], dt.bfloat16)
        C.xgt2 = C.xgt2_pool.tile([128, 4, MC, S2], dt.bfloat16)
        nc.sync.dma_start(
            out=C.xgt2[:, :, :, :],
            in_=C.xg2[:, tok_base + j:, :, :][:, bass.ds(w1, 4), :, :],
        )
        nc.sync.dma_start(
            out=C.xgt1[:, :, :, :],
            in_=C.xg1[:, tok_base + j:, :, :][:, bass.ds(w1, 4), :, :],
        )
    for u in (1, 0):
        if u == 1:
            gA, gB, xgt, su = C.g2A, C.g2B, C.xgt2, S2
        else:
            gA, gB, xgt, su = C.g1A, C.g1B, C.xgt1, S1
        for grp, gps in ((0, gA), (1, gB)):
            for m8 in range(8):
                nc.tensor.matmul(
                    gps[:, m8, 0:su],
                    C.ident_sb[:, :],
                    xgt[:, j % 4, grp * 8 + m8, :],
                    start=(m8 == 0),
                    stop=False,
                )


def _emit_scan_main(C, j, tok_base, w1, hdst_view):
    """Whh matmuls + cell update for both units; h store every 2nd step.
    hdst_view: dram view [128, HK, nt, S] receiving bf16 h at index w+j."""
    nc, bass, mybir = C.nc, C.bass, C.mybir
    A = mybir.ActivationFunctionType

    for u in (1, 0):  # big unit first
        if u == 1:
            ssl, su, whh = slice(S1, S), S2, C.whh2_sb
            gA, gB, act = C.g2A, C.g2B, C.act2
            tg, t1s, t2s, tcv = C.tg2, C.t12, C.t22, C.tc2
        else:
            ssl, su, whh = slice(0, S1), S1, C.whh1_sb
            gA, gB, act = C.g1A, C.g1B, C.act1
            tg, t1s, t2s, tcv = C.tg1, C.t11, C.t21, C.tc1

        # accumulate Whh@h in fp8 DoubleRow on top of the injected xg
        for grp, gps in ((0, gA), (1, gB)):
            for m8 in range(8):
                for kp in range(2):
                    nc.tensor.matmul(
                        gps[:, m8, 0:su],
                        whh[:, kp, :, grp * 8 + m8, :],
                        C.h8[:, 2 * kp:2 * kp + 2, ssl],
                        start=False,
                        stop=(m8 == 7 and kp == 1),
                        perf_mode=mybir.MatmulPerfMode.DoubleRow,
                    )
        # group A done -> tanh(g) [chunks 0:4], sigmoid(i) [4:8]
        nc.scalar.activation(tg[:, :, :], gA[:, 0:4, 0:su], A.Tanh, scale=1.0 / WS)
        nc.scalar.activation(act[:, 0:4, :], gA[:, 4:8, 0:su], A.Sigmoid, scale=1.0 / WS)
        # group B done -> sigmoid(f,o) [8:16]
        nc.scalar.activation(act[:, 4:12, :], gB[:, :, 0:su], A.Sigmoid, scale=1.0 / WS)
        # c = sig(f)*c + sig(i)*tanh(g);  h = sig(o)*tanh(c)
        nc.vector.tensor_mul(t1s[:, :, :], act[:, 0:4, :], tg[:, :, :])
        nc.vector.tensor_mul(t2s[:, :, :], act[:, 4:8, :], C.c_sb[:, :, ssl])
        nc.vector.tensor_add(C.c_sb[:, :, ssl], t1s[:, :, :], t2s[:, :, :])
        nc.scalar.activation(tcv[:, :, :], C.c_sb[:, :, ssl], A.Tanh)
        nc.vector.tensor_mul(C.h8[:, :, ssl], act[:, 8:12, :], tcv[:, :, :])
        # bf16 h for the handoff / output, from the fp32 operands (NOT from
        # the fp8 state - fp8 noise here would leak into L1 and the output);
        # 4 rotating t-slots so the paired store never stalls the queues
        nc.vector.tensor_mul(C.hbf[:, :, j % 4, ssl], act[:, 8:12, :], tcv[:, :, :])

    if j % 2 == 1:  # store two steps of h per DMA
        sl = (j - 1) % 4
        nc.gpsimd.dma_start(
            out=hdst_view[:, :, j - 1:, :][:, :, bass.ds(w1, 2), :],
            in_=C.hbf[:, :, sl:sl + 2, :],
        )


def build_nc(n_cores=NCORE, unroll=False):
    import bass_rust
    import concourse.bass as bass
    import concourse.mybir as mybir
    from concourse import tile
    from contextlib import ExitStack

    dt = mybir.dt
    nc = bass.Bass("TRN2", target_bir_lowering=False, debug=False,
                   num_devices=(1 if unroll else n_cores))

    C = _Ctx()
    C.nc, C.bass, C.mybir = nc, bass, mybir

    C.xT = nc.dram_tensor("xT", [DK, 128, T, S], dt.bfloat16, kind="ExternalInput").ap()
    w_in = {}
    for nm in ("A1", "A2", "B1", "B2"):
        w_in[f"wih{nm}"] = nc.dram_tensor(f"wih{nm}", [128, DK, MC, 128], dt.bfloat16, kind="ExternalInput").ap()
        w_in[f"whh{nm}"] = nc.dram_tensor(f"whh{nm}", [128, 2, 2, MC, 128], dt.float8e4, kind="ExternalInput").ap()
        w_in[f"bias{nm}"] = nc.dram_tensor(f"bias{nm}", [128, MC], dt.float32, kind="ExternalInput").ap()
    ident = nc.dram_tensor("ident", [128, 128], dt.bfloat16, kind="ExternalInput").ap()
    flag = nc.dram_tensor("flag", [1, 4], dt.int32, kind="ExternalInput").ap()
    out = nc.dram_tensor("out", [HK, 128, T, S], dt.bfloat16, kind="ExternalOutput").ap()

    C.xg1 = nc.dram_tensor("xg1", [128, T, MC, S1], dt.bfloat16).ap()
    C.xg2 = nc.dram_tensor("xg2", [128, T, MC, S2], dt.bfloat16).ap()
    hT = nc.dram_tensor("hT", [HK, 128, T, S], dt.bfloat16).ap()
    # own h reversed: hTr_A = reverse(hT[TH:]) (global tokens 255..128),
    # hTr_B = reverse(hT[:TH]); A ships first (the partner needs it first).
    hTr_A = nc.dram_tensor("hTr_A", [HK, 128, TH, S], dt.bfloat16).ap()
    hTr_B = nc.dram_tensor("hTr_B", [HK, 128, TH, S], dt.bfloat16).ap()
    gathR_A = nc.dram_tensor("gathR_A", [2, HK, 128, TH, S], dt.bfloat16).ap()
    gathR_B = nc.dram_tensor("gathR_B", [2, HK, 128, TH, S], dt.bfloat16).ap()

    SafeTC = _safe_tc(tile, bass_rust)
    groups = [[2 * k, 2 * k + 1] for k in range(max(n_cores // 2, 1))]

    with SafeTC(nc) as tc, ExitStack() as ctx:
        cpool = ctx.enter_context(tc.tile_pool(name="const", bufs=1))
        C.wih1_sb = cpool.tile([128, DK, MC, 128], dt.bfloat16, name="wih1_sb")
        C.wih2_sb = cpool.tile([128, DK, MC, 128], dt.bfloat16, name="wih2_sb")
        C.whh1_sb = cpool.tile([128, 2, 2, MC, 128], dt.float8e4, name="whh1_sb")
        C.whh2_sb = cpool.tile([128, 2, 2, MC, 128], dt.float8e4, name="whh2_sb")
        C.bias1_sb = cpool.tile([128, MC], dt.float32, name="bias1_sb")
        C.bias2_sb = cpool.tile([128, MC], dt.float32, name="bias2_sb")
        C.ident_sb = cpool.tile([128, 128], dt.bfloat16, name="ident_sb")
        flag_sb = cpool.tile([1, 4], dt.int32, name="flag_sb")

        def load_layer(nm1, nm2):
            for sb, dr in [(C.wih1_sb, w_in[f"wih{nm1}"]), (C.wih2_sb, w_in[f"wih{nm2}"]),
                           (C.whh1_sb, w_in[f"whh{nm1}"]), (C.whh2_sb, w_in[f"whh{nm2}"]),
                           (C.bias1_sb, w_in[f"bias{nm1}"]), (C.bias2_sb, w_in[f"bias{nm2}"])]:
                nc.sync.dma_start(out=sb[...], in_=dr[...])

        load_layer("A1", "A2")
        nc.sync.dma_start(out=C.ident_sb[...], in_=ident[...])
        nc.sync.dma_start(out=flag_sb[...], in_=flag[...])

        if unroll:
            C.vb = 1
        else:
            tmp = nc.alloc_registers("vb_r")
            nc.regs_load(tmp, flag_sb[0:1, 1:2])
            C.vb = nc.snap(tmp, donate=True, min_val=0, max_val=1)

        C.pending_st = []
        spool = ctx.enter_context(tc.tile_pool(name="state", bufs=1))
        C.h8 = spool.tile([128, HK, S], dt.float8e4, name="h8")
        C.hbf = spool.tile([128, HK, 4, S], dt.bfloat16, name="hbf")
        C.c_sb = spool.tile([128, HK, S], dt.float32, name="c_sb")

        def gview(gh, slot):
            """[128, HK, TH, S] view of gath half gh at slot (static)."""
            return gh[slot].rearrange("k p t j -> p k t j")

        def hview(ht):
            return ht.rearrange("k p t j -> p k t j")

        for lay in range(2):
            nm1, nm2 = ("A1", "A2") if lay == 0 else ("B1", "B2")
            with ExitStack() as phase:
                C.rhs_pool = phase.enter_context(tc.tile_pool(name=f"rhs{lay}", bufs=2))
                C.ip_ps_pool = phase.enter_context(tc.tile_pool(name=f"ipps{lay}", bufs=4, space="PSUM"))
                C.stg1_pool = phase.enter_context(tc.tile_pool(name=f"s1p{lay}", bufs=2))
                C.stg2_pool = phase.enter_context(tc.tile_pool(name=f"s2p{lay}", bufs=2))
                C.xgt1_pool = phase.enter_context(tc.tile_pool(name=f"x1p{lay}", bufs=4))
                C.xgt2_pool = phase.enter_context(tc.tile_pool(name=f"x2p{lay}", bufs=4))
                C.prt_pool = phase.enter_context(tc.tile_pool(name=f"prt{lay}", bufs=2))
                gpool = phase.enter_context(tc.tile_pool(name=f"g{lay}", bufs=1, space="PSUM"))
                apool = phase.enter_context(tc.tile_pool(name=f"act{lay}", bufs=1))

                C.g1A = gpool.tile([128, 8, 64], dt.float32, name=f"g1A{lay}", space="PSUM")
                C.g1B = gpool.tile([128, 8, 64], dt.float32, name=f"g1B{lay}", space="PSUM")
                C.g2A = gpool.tile([128, 8, 64], dt.float32, name=f"g2A{lay}", space="PSUM")
                C.g2B = gpool.tile([128, 8, 64], dt.float32, name=f"g2B{lay}", space="PSUM")
                C.act1 = apool.tile([128, 12, S1], dt.float32, name=f"a1{lay}")
                C.act2 = apool.tile([128, 12, S2], dt.float32, name=f"a2{lay}")
                C.tg1 = apool.tile([128, HK, S1], dt.float32, name=f"tg1{lay}")
                C.tg2 = apool.tile([128, HK, S2], dt.float32, name=f"tg2{lay}")
                C.t11 = apool.tile([128, HK, S1], dt.float32, name=f"t11{lay}")
                C.t12 = apool.tile([128, HK, S2], dt.float32, name=f"t12{lay}")
                C.t21 = apool.tile([128, HK, S1], dt.float32, name=f"t21{lay}")
                C.t22 = apool.tile([128, HK, S2], dt.float32, name=f"t22{lay}")
                C.tc1 = apool.tile([128, HK, S1], dt.float32, name=f"tc1{lay}")
                C.tc2 = apool.tile([128, HK, S2], dt.float32, name=f"tc2{lay}")

                if lay == 1:
                    load_layer(nm1, nm2)
                nc.vector.memset(C.h8[:, :, :], 0.0)
                nc.vector.memset(C.hbf[:, :, :, :], 0.0)
                nc.vector.memset(C.c_sb[:, :, :], 0.0)

                # segments: (nsteps, tok_base, l1 normal view, l1 rev view,
                #            h-store view, do_ip)
                if lay == 0:
                    segs = [
                        (T - LEAD_T, 0, None, hview(hT), True),
                        (LEAD_T, T - LEAD_T, None,
                         hview(hT)[:, :, T - LEAD_T:, :], False),
                    ]
                else:
                    # partner h (already reversed to global 255-tau order)
                    C.gRA_v = gathR_A[bass.ds(C.vb, 1), :, :, :, :][0] \
                        .rearrange("k p t j -> p k t j")
                    C.gRB_v = gathR_B[bass.ds(C.vb, 1), :, :, :, :][0] \
                        .rearrange("k p t j -> p k t j")
                    C.gran = {}
                    segs = [
                        (96, 0, hview(hT)[:, :, LEAD_T:, :], hview(out), True),
                        (128, 96, hview(hT)[:, :, TH:, :],
                         hview(out)[:, :, 96:, :], True),
                        (LEAD_T, T - LEAD_T, None,
                         hview(out)[:, :, T - LEAD_T:, :], False),
                    ]

                # IP lead groups (static indices; unsliced views)
                if lay == 0:
                    C.l1_src = None
                else:
                    C.l1_src = hview(hT)
                    _emit_granule_load(C, 0)
                for g in range(GLEAD):
                    base = g * GT
                    for j in range(GT):
                        _emit_ip_subchunk(C, j, 0, base, base * S1, base * S2)
                        _emit_ip_sts(C)

                for (nsteps, tok_base, l1s, hv, do_ip) in segs:
                    C.l1_src = l1s
                    for i in range(0, nsteps, GT):
                        for j in range(GT):
                            _emit_scan_pre(C, j, tok_base, i)
                            if do_ip:
                                _emit_ip_subchunk(
                                    C, j, tok_base + LEAD_T, i, i * S1, i * S2)
                            _emit_scan_main(C, j, tok_base, i, hv)
                            if do_ip:
                                _emit_ip_sts(C)

                if lay == 0:
                    # reverse own h locally (per k-chunk: 3-dim DMAs), then
                    # AllGather; the A half (global 255..128) ships first
                    # because the partner's L1-IP consumes it first.
                    for kc in range(HK):
                        nc.sync.dma_start(
                            out=hTr_A[kc, :, :, :],
                            in_=_rev_t(hT[kc, :, TH:, :], 1),
                        )
                    for kc in range(HK):
                        nc.scalar.dma_start(
                            out=hTr_B[kc, :, :, :],
                            in_=_rev_t(hT[kc, :, 0:TH, :], 1),
                        )
                    if unroll:
                        for g, hsrc in ((gathR_A, hTr_A), (gathR_B, hTr_B)):
                            for v in range(2):
                                nc.sync.dma_start(out=g[v, :, :, :, :], in_=hsrc[...])
                    else:
                        nc.gpsimd.collective_compute(
                            "AllGather", mybir.AluOpType.bypass,
                            replica_groups=groups,
                            ins=[hTr_A[...]], outs=[gathR_A[0:2, :, :, :, :]],
                        )
                        nc.gpsimd.collective_compute(
                            "AllGather", mybir.AluOpType.bypass,
                            replica_groups=groups,
                            ins=[hTr_B[...]], outs=[gathR_B[0:2, :, :, :, :]],
                        )
    _split_waits(nc, mybir)
    return nc


# ---------------- host-side data prep ----------------

GATE_PERM = (2, 0, 1, 3)  # new block order [g, i, f, o] from [i, f, g, o]


def _reorder_gates(W):
    """W: (4H, ...) -> gate blocks reordered to [g, i, f, o]."""
    blocks = W.reshape(4, H, *W.shape[1:])
    return np.concatenate([blocks[p] for p in GATE_PERM], axis=0)


def _wih_tiles(W, swap_k_halves=False):
    """(2048, K) fp32 -> [128(kp), K/128, MC, 128(mp)] bf16, x32 scaled.
    swap_k_halves: put columns 512:1024 first (bwd cores' L1 weights: the
    rhs always loads own-direction h into kc 0:4)."""
    Wr = _reorder_gates(W) * WS
    if swap_k_halves:
        Wr = np.concatenate([Wr[:, H:], Wr[:, :H]], axis=1)
    M, K = Wr.shape
    t = Wr.reshape(MC, 128, K // 128, 128)       # [mc, mp, kc, kp]
    return np.ascontiguousarray(t.transpose(3, 2, 0, 1)).astype(BF16)


def _whh_dr_tiles(W):
    """(2048, 512) fp32 -> [128(kp), 2(pair), 2(j), MC, 128(mp)] fp8, x32."""
    Wr = _reorder_gates(W) * WS
    t = Wr.reshape(MC, 128, 4, 128)              # [mc, mp, kchunk, kp]
    t = t.reshape(MC, 128, 2, 2, 128)            # [mc, mp, pair, j, kp]
    return np.ascontiguousarray(t.transpose(4, 2, 3, 0, 1)).astype(FP8)


def _bias_tiles(b):
    br = _reorder_gates(b) * WS
    return np.ascontiguousarray(br.reshape(MC, 128).T.astype(np.float32))


def _core_inputs(u1, u2):
    """u1 (T,S1,D), u2 (T,S2,D) fp32 local time -> xT (DK, 128, T, S) bf16."""
    x48 = np.concatenate([u1, u2], axis=1)          # (T, S, D)
    xt = x48.transpose(2, 0, 1)                     # (D, T, S)
    return np.ascontiguousarray(xt.reshape(DK, 128, T, S)).astype(BF16)


def _prep_inputs(inputs):
    U = np.asarray(inputs["U"], np.float32)            # (T, B, D)
    qmask = np.asarray(inputs["qmask"], np.float32)    # (B, T, P)
    U_bt = U.transpose(1, 0, 2)
    mask = qmask > 0
    pos = np.cumsum(mask.astype(np.int64), axis=1) - 1

    parties = np.zeros((P, B, T, D), np.float32)
    for p in range(P):
        b_idx, t_idx = np.nonzero(mask[:, :, p])
        parties[p, b_idx, pos[b_idx, t_idx, p]] = U_bt[b_idx, t_idx]
    partiesM = parties.reshape(P * B, T, D).transpose(1, 0, 2)  # (T, 128, D)

    def wset(stack, lay, d):
        return (
            _wih_tiles(np.asarray(inputs[f"{stack}_Wih{lay}"][d], np.float32),
                       swap_k_halves=(lay == 1 and d == 1)),
            _whh_dr_tiles(np.asarray(inputs[f"{stack}_Whh{lay}"][d], np.float32)),
            _bias_tiles(np.asarray(inputs[f"{stack}_b{lay}"][d], np.float32)),
        )

    wsets = {(st, la, d): wset(st, la, d)
             for st in ("rnn", "rnnp") for la in (0, 1) for d in (0, 1)}
    ident_np = np.eye(128, dtype=BF16)

    in_maps = []
    for c in range(NCORE):
        k, d = c // 2, c % 2
        u1 = U[:, 16 * k:16 * k + 16, :]
        u2 = partiesM[:, 32 * k:32 * k + 32, :]
        if d == 1:
            u1, u2 = u1[::-1], u2[::-1]
        m = {
            "xT": _core_inputs(u1, u2),
            "ident": ident_np,
            # [unused, partner-slot, cond_slot0=(slot==0), cond_slot1]
            "flag": np.array([[0, 1 - d, d, 1 - d]], np.int32),
        }
        for la, nm in ((0, "A"), (1, "B")):
            for ui, st in ((1, "rnn"), (2, "rnnp")):
                wih, whh, bias = wsets[(st, la, d)]
                m[f"wih{nm}{ui}"] = wih
                m[f"whh{nm}{ui}"] = whh
                m[f"bias{nm}{ui}"] = bias
        in_maps.append(m)
    return in_maps, mask, pos


def _assemble(results, mask, pos):
    # per-core out: (HK, 128, T, S) bf16; feature dim on (HK,128)=512
    o = []
    for c in range(NCORE):
        oc = np.asarray(results[c]["out"]).astype(np.float32)
        oc = oc.reshape(H, T, S).transpose(1, 2, 0)    # (T, S, 512) local time
        if c % 2 == 1:
            oc = oc[::-1]
        o.append(oc)

    U_s = np.zeros((T, B, 2 * H), np.float32)
    E = np.zeros((P, B, T, 2 * H), np.float32)
    for k in range(4):
        fwd, bwd = o[2 * k], o[2 * k + 1]
        U_s[:, 16 * k:16 * k + 16, 0:H] = fwd[:, 0:S1]
        U_s[:, 16 * k:16 * k + 16, H:2 * H] = bwd[:, 0:S1]
        for i in range(S2):
            ms = 32 * k + i
            p, b = divmod(ms, B)
            E[p, b, :, 0:H] = fwd[:, S1 + i]
            E[p, b, :, H:2 * H] = bwd[:, S1 + i]

    U_p = np.zeros((B, T, 2 * H), np.float32)
    for p in range(P):
        idx = np.clip(pos[:, :, p], 0, T - 1)
        gathered = np.take_along_axis(E[p], idx[:, :, None], axis=1)
        U_p = np.where(mask[:, :, p][:, :, None], gathered, U_p)
    U_p = U_p.transpose(1, 0, 2)
    return np.concatenate([U_s, U_p], axis=-1).astype(np.float32)


def _get_compiled():
    if "nc" not in _CACHE:
        _CACHE["nc"] = build_nc()
    return _CACHE["nc"]


def kernel(**inputs):
    from concourse.bass_utils import run_bass_kernel_spmd

    nc = _get_compiled()
    in_maps, mask, pos = _prep_inputs(inputs)
    trace = bool(int(os.environ.get("KERNEL_TRACE", "0")))
    res = run_bass_kernel_spmd(nc, in_maps, list(range(NCORE)), trace=trace)
    _CACHE["last_exec_time_ns"] = res.exec_time_ns
    return _assemble(res.results, mask, pos)
